# revision 20
# baseline (speedup 1.0000x reference)
"""Trainium2 Bass kernel for nn_NonLocAtt (non-local attention block).

Reference computation (per batch b):
    H_kv = w_kv @ H[b]            # [32, 8192]   (k = t*1024 + n)
    h_q  = w_q  @ h[b]            # [32, 1024]
    a[t, n, m] = h_q[:, n] . H_kv[:, t, m]     # scores
    a[t, m, m] = 0                             # zero per-t diagonal
    att = softmax over rows (t, n) of a        # [8192, 1024]
    h_v = H_kv_flat @ att                      # [32, 1024]
    y   = h[b] + w_o @ h_v                     # [64, 1024]
    return (y, att)

Sharding: 8 cores = 4 batches x 2 column-halves (columns m are fully
independent: softmax is over rows, h_v contracts rows).  One SPMD program
serves all cores via the "rotation trick": each core's inputs are
pre-rotated along n by its column offset m0, so the per-t diagonal always
lands in local n-chunks 0..3 at a program-fixed position; the host
un-rotates att rows when assembling the full output.

On-chip layout: rows (t, n) on partitions (tiles of 128), columns m on the
free axis.  Pass 1 computes exp(scores) into a 16 MB SBUF tile; a
ones-augmented H_kv^T matmul accumulates h_v and the softmax column sums
together.  Pass 2 normalizes (DVE/GPSIMD split) and streams att to HBM.
Matmuls use float32r (full-rate single-pass fp32) where N >= 256.
"""

import numpy as np

B, C, T, Hh, Ww = 4, 64, 8, 32, 32
HID = 32
N = Hh * Ww          # 1024
TN = T * N           # 8192
MH = N // 2          # 512 columns per core
MC = 2               # column chunks per core (pipeline pass1 vs pass2)
W = MH // MC         # chunk width
NCORES = 8

_cache = {}


def _build_program():
    import concourse.bass as bass
    import concourse.tile as tile
    from concourse import bacc, mybir

    f32 = mybir.dt.float32
    f32r = mybir.dt.float32r
    Exp = mybir.ActivationFunctionType.Exp

    nc = bacc.Bacc("TRN2", target_bir_lowering=False, debug=False,
                   num_devices=NCORES)

    # Per-core inputs (host pre-shards / pre-rotates).
    hf_d = nc.dram_tensor("hf", [C, TN], f32, kind="ExternalInput")     # rotated
    hs_d = nc.dram_tensor("hs", [C, T * MH], f32, kind="ExternalInput")  # unrotated m-slice
    hb_d = nc.dram_tensor("hb", [C, N], f32, kind="ExternalInput")      # rotated
    hbs_d = nc.dram_tensor("hbs", [C, MH], f32, kind="ExternalInput")   # unrotated m-slice
    wqt_d = nc.dram_tensor("wqt", [C, HID], f32, kind="ExternalInput")
    wkvt_d = nc.dram_tensor("wkvt", [C, HID], f32, kind="ExternalInput")
    wot_d = nc.dram_tensor("wot", [HID, C], f32, kind="ExternalInput")
    nm_d = nc.dram_tensor("nmask", [128, 128], f32, kind="ExternalInput")  # 1 - I

    att_d = nc.dram_tensor("att", [TN, MH], f32, kind="ExternalOutput")
    y_d = nc.dram_tensor("y", [C, MH], f32, kind="ExternalOutput")

    # DRAM view of att for the tiled writes: rows k = t*1024 + jl*128 + p.
    att_r = att_d.ap().rearrange("(t j p) m -> t p j m", t=T, j=8, p=128)

    with tile.TileContext(nc) as tc:
        with (
            tc.tile_pool(name="const", bufs=1) as cp,
            tc.tile_pool(name="load", bufs=2) as lp,
            tc.tile_pool(name="bigp", bufs=1) as bp,
            tc.tile_pool(name="stage", bufs=1, space=bass.MemorySpace.PSUM) as stp,
            tc.tile_pool(name="scp", bufs=3, space=bass.MemorySpace.PSUM) as scp,
            tc.tile_pool(name="hvp", bufs=1, space=bass.MemorySpace.PSUM) as hvp,
            tc.tile_pool(name="miscp", bufs=1, space=bass.MemorySpace.PSUM) as mp,
        ):
            # ---------------- persistent SBUF tiles ----------------
            wqt = cp.tile([C, HID], f32, tag="wqt")
            wkvt = cp.tile([C, HID], f32, tag="wkvt")
            wot = cp.tile([HID, C], f32, tag="wot")
            nmask = cp.tile([128, 128], f32, tag="nmask")
            hb = cp.tile([C, N], f32, tag="hb")
            hbs = cp.tile([C, MH], f32, tag="hbs")
            hq = cp.tile([HID, N], f32r, tag="hq")
            hkvs = cp.tile([HID, T * MH], f32r, tag="hkvs")
            hkvt = cp.tile([128, 64 * (HID + 1)], f32r, tag="hkvt")
            ones1 = cp.tile([1, 128], f32, tag="ones1")
            one_1 = cp.tile([1, 1], f32, tag="one_1")
            big = bp.tile([128, 64 * MH], f32r, tag="big")

            nc.sync.dma_start(wqt[:], wqt_d.ap())
            nc.sync.dma_start(wkvt[:], wkvt_d.ap())
            nc.sync.dma_start(wot[:], wot_d.ap())
            nc.sync.dma_start(nmask[:], nm_d.ap())
            nc.sync.dma_start(hb[:], hb_d.ap())
            nc.sync.dma_start(hbs[:], hbs_d.ap())
            nc.gpsimd.memset(ones1[:], 1.0)
            nc.gpsimd.memset(one_1[:], 1.0)

            # ---------------- prologue: h_q ----------------
            for i in range(2):
                st = stp.tile([HID, 512], f32, tag="stage")
                nc.tensor.matmul(st[:], wqt[:],
                                 hb[:, i * 512:(i + 1) * 512])
                nc.scalar.copy(hq[:, i * 512:(i + 1) * 512], st[:])

            # ---------------- prologue: H_kv over the m-slice ----------------
            for q in range(4):
                hsq = lp.tile([C, 1024], f32, tag="hsq")
                nc.sync.dma_start(hsq[:], hs_d.ap()[:, q * 1024:(q + 1) * 1024])
                for tt in range(2):
                    t = 2 * q + tt
                    st = stp.tile([HID, 512], f32, tag="stage")
                    nc.tensor.matmul(st[:], wkvt[:],
                                     hsq[:, tt * 512:(tt + 1) * 512])
                    nc.scalar.copy(hkvs[:, t * 512:(t + 1) * 512], st[:])

            # ---------------- prologue: H_kv^T (augmented) ----------------
            # 8 chunks of [128 k, 32 d + ones col] per stage tile; the ones
            # column is produced in PSUM by a K=1 outer-product matmul so the
            # whole augmented group is written to SBUF by ONE contiguous
            # (f32r-rounding) copy.
            GW = 8 * (HID + 1)  # 264
            for q8 in range(8):
                hfq = lp.tile([C, 1024], f32, tag="hfq")
                nc.sync.dma_start(hfq[:],
                                  hf_d.ap()[:, q8 * 1024:(q8 + 1) * 1024])
                kvst = stp.tile([128, GW], f32, tag="stage")
                for cl in range(8):
                    o = cl * (HID + 1)
                    nc.tensor.matmul(
                        kvst[:, o:o + HID],
                        hfq[:, cl * 128:(cl + 1) * 128],
                        wkvt[:],
                        start=(cl == 0), stop=False)
                    nc.tensor.matmul(
                        kvst[:, o + HID:o + HID + 1],
                        ones1[:], one_1[:],
                        start=False, stop=(cl == 7))
                nc.vector.tensor_copy(hkvt[:, q8 * GW:(q8 + 1) * GW], kvst[:])

            # ---------------- main passes ----------------
            bcast_sb, hv_ps = {}, {}

            def emit_scores(c, i):
                t, jl = divmod(i, 8)
                sc = scp.tile([128, W], f32, tag="sc")
                nc.tensor.matmul(
                    sc[:],
                    hq[:, jl * 128:(jl + 1) * 128],
                    hkvs[:, t * MH + c * W: t * MH + (c + 1) * W])
                blk_lo = c * W // 128  # first jl with diag in this chunk
                if blk_lo <= jl < blk_lo + W // 128:
                    sub = (jl - blk_lo) * 128
                    nc.vector.tensor_mul(sc[:, sub:sub + 128],
                                         sc[:, sub:sub + 128], nmask[:])
                nc.scalar.activation(
                    big[:, i * MH + c * W: i * MH + (c + 1) * W], sc[:], Exp)
                return sc

            def emit_hv(c, i):
                nc.tensor.matmul(
                    hv_ps[c][:],
                    hkvt[:, i * (HID + 1):(i + 1) * (HID + 1)],
                    big[:, i * MH + c * W: i * MH + (c + 1) * W],
                    start=(i == 0), stop=(i == 63))

            def emit_pass1(c, norm_c=None):
                hv_ps[c] = hvp.tile([HID + 1, W], f32, tag=f"hv{c}",
                                    name=f"hv{c}")
                emit_scores(c, 0)
                emit_scores(c, 1)
                for i in range(64):
                    if i + 2 < 64:
                        emit_scores(c, i + 2)
                    emit_hv(c, i)
                    if norm_c is not None:
                        emit_norm(norm_c, i)
                        if i % 8 == 7:
                            emit_att_dma(norm_c, i // 8)

            def emit_inter(c):
                ssum = cp.tile([1, W], f32, tag="ssum", name=f"ssum{c}")
                nc.vector.tensor_copy(ssum[:], hv_ps[c][HID:HID + 1, :])
                recip = cp.tile([1, W], f32, tag="recip")
                nc.vector.reciprocal_approx_fast(recip[:], ssum[:])
                bc = mp.tile([128, W], f32, tag="bc")
                nc.tensor.matmul(bc[:], ones1[:], recip[:])
                bcast_sb[c] = cp.tile([128, W], f32, tag="bcast",
                                      name=f"bcast{c}")
                nc.vector.tensor_copy(bcast_sb[c][:], bc[:])

            def emit_norm(c, i):
                sl = big[:, i * MH + c * W: i * MH + (c + 1) * W]
                eng = nc.vector if i % 3 != 2 else nc.gpsimd
                eng.tensor_mul(sl, sl, bcast_sb[c][:])

            def emit_att_dma(c, t):
                big_r = big[:].rearrange("p (i m) -> p i m", m=MH)
                nc.sync.dma_start(
                    att_r[t][:, :, c * W:(c + 1) * W],
                    big_r[:, t * 8:(t + 1) * 8, c * W:(c + 1) * W].bitcast(f32))

            def emit_y(c):
                hv_sb = cp.tile([HID, W], f32, tag="hv_sb")
                nc.vector.tensor_mul(hv_sb[:], hv_ps[c][:HID, :],
                                     bcast_sb[c][:HID, :])
                yac = mp.tile([C, W], f32, tag="y")
                nc.tensor.matmul(yac[:], wot[:], hv_sb[:])
                y_sb = cp.tile([C, W], f32, tag="y_sb")
                nc.vector.tensor_add(y_sb[:], yac[:],
                                     hbs[:, c * W:(c + 1) * W])
                nc.sync.dma_start(y_d.ap()[:, c * W:(c + 1) * W], y_sb[:])

            if MC == 1:
                emit_pass1(0)
                emit_inter(0)
                for i in range(64):
                    emit_norm(0, i)
                    if i % 8 == 7:
                        emit_att_dma(0, i // 8)
                emit_y(0)
            else:
                emit_pass1(0)
                emit_inter(0)
                emit_pass1(1, norm_c=0)
                emit_y(0)
                emit_inter(1)
                for i in range(64):
                    emit_norm(1, i)
                    if i % 8 == 7:
                        emit_att_dma(1, i // 8)
                emit_y(1)

    nc.compile()
    return nc


def _shard_inputs(H, h, w_q, w_kv, w_o):
    """Build the 8 per-core input dicts (with the n-rotation trick)."""
    Hf = np.ascontiguousarray(H.reshape(B, C, TN), dtype=np.float32)
    hb = np.ascontiguousarray(h.reshape(B, C, N), dtype=np.float32)
    wqt = np.ascontiguousarray(w_q.T, dtype=np.float32)
    wkvt = np.ascontiguousarray(w_kv.T, dtype=np.float32)
    wot = np.ascontiguousarray(w_o.T, dtype=np.float32)
    nmask = np.ascontiguousarray(1.0 - np.eye(128), dtype=np.float32)

    in_maps = []
    for core in range(NCORES):
        b, hh = divmod(core, 2)
        m0 = hh * MH
        Hb = Hf[b].reshape(C, T, N)
        hf_rot = np.roll(Hb, -m0, axis=2).reshape(C, TN)
        hs = Hb[:, :, m0:m0 + MH].reshape(C, T * MH)
        hb_rot = np.roll(hb[b], -m0, axis=1)
        hbs = hb[b][:, m0:m0 + MH]
        in_maps.append({
            "hf": np.ascontiguousarray(hf_rot),
            "hs": np.ascontiguousarray(hs),
            "hb": np.ascontiguousarray(hb_rot),
            "hbs": np.ascontiguousarray(hbs),
            "wqt": wqt, "wkvt": wkvt, "wot": wot, "nmask": nmask,
        })
    return in_maps


def _assemble(results):
    """Gather per-core outputs into full (y, att), undoing the rotation."""
    y_full = np.empty((B, C, N), dtype=np.float32)
    att_full = np.empty((B, TN, N), dtype=np.float32)
    for core in range(NCORES):
        b, hh = divmod(core, 2)
        m0 = hh * MH
        r = results[core]
        y_full[b][:, m0:m0 + MH] = r["y"]
        a = r["att"].reshape(T, N, MH)
        if m0:
            a = np.roll(a, m0, axis=1)
        att_full[b][:, m0:m0 + MH] = a.reshape(TN, MH)
    return y_full.reshape(B, C, Hh, Ww), att_full


def _run(in_maps, trace=False, **kw):
    from concourse.bass_utils import run_bass_kernel_spmd
    if "prog" not in _cache:
        _cache["prog"] = _build_program()
    nc = _cache["prog"]
    return run_bass_kernel_spmd(nc, in_maps, core_ids=list(range(NCORES)),
                                trace=trace, **kw)


def kernel(H, h, w_q, w_kv, w_o):
    in_maps = _shard_inputs(np.asarray(H), np.asarray(h), np.asarray(w_q),
                            np.asarray(w_kv), np.asarray(w_o))
    res = _run(in_maps)
    return _assemble(res.results)


# revision 22
# speedup vs baseline: 1.7477x; 1.7477x over previous
"""Trainium2 Bass kernel for nn_NonLocAtt (non-local attention block).

Reference computation (per batch b):
    H_kv = w_kv @ H[b]            # [32, 8192]   (k = t*1024 + n)
    h_q  = w_q  @ h[b]            # [32, 1024]
    a[t, n, m] = h_q[:, n] . H_kv[:, t, m]     # scores
    a[t, m, m] = 0                             # zero per-t diagonal
    att = softmax over rows (t, n) of a        # [8192, 1024]
    h_v = H_kv_flat @ att                      # [32, 1024]
    y   = h[b] + w_o @ h_v                     # [64, 1024]
    return (y, att)

Sharding: 8 cores = 4 batches x 2 column-halves (columns m are fully
independent: softmax is over rows, h_v contracts rows).  One SPMD program
serves all cores via the "rotation trick": each core's inputs are
pre-rotated along n by its column offset m0, so the per-t diagonal always
lands in local n-chunks 0..3 at a program-fixed position; the host
un-rotates att rows when assembling the full output.

On-chip layout: rows (t, n) on partitions (tiles of 128), columns m on the
free axis.  The hot matmuls run in fp16 (fp32 matmuls lower to 2 half-rate
passes on TRN2 = 4x slower; fp16 keeps 10 mantissa bits, ~5e-4).  Pass 1
computes exp(scores) (fp16) into an 8 MB SBUF tile; a ones-augmented
H_kv^T matmul accumulates h_v and the softmax column sums together in one
PSUM group.  Pass 2 normalizes in fp16 on DVE (2x mode) and streams att
out via cast-to-f32 SWDGE DMA.  Two column chunks pipeline pass 1 of one
chunk against pass 2 of the other.
"""

import numpy as np

B, C, T, Hh, Ww = 4, 64, 8, 32, 32
HID = 32
N = Hh * Ww          # 1024
TN = T * N           # 8192
MH = N // 2          # 512 columns per core
MC = 2               # column chunks per core (pipeline pass1 vs pass2)
W = MH // MC         # chunk width (256)
NCORES = 8

_cache = {}


def _build_program():
    import concourse.bass as bass
    import concourse.tile as tile
    from concourse import bacc, mybir

    f32 = mybir.dt.float32
    f16 = mybir.dt.float16
    Exp = mybir.ActivationFunctionType.Exp

    nc = bacc.Bacc("TRN2", target_bir_lowering=False, debug=False,
                   num_devices=NCORES)

    # Per-core inputs (host pre-shards / pre-rotates).
    hf_d = nc.dram_tensor("hf", [C, TN], f32, kind="ExternalInput")     # rotated
    hs_d = nc.dram_tensor("hs", [C, T * MH], f32, kind="ExternalInput")  # unrotated m-slice
    hb_d = nc.dram_tensor("hb", [C, N], f32, kind="ExternalInput")      # rotated
    hbs_d = nc.dram_tensor("hbs", [C, MH], f32, kind="ExternalInput")   # unrotated m-slice
    wqt_d = nc.dram_tensor("wqt", [C, HID], f32, kind="ExternalInput")
    wkvt_d = nc.dram_tensor("wkvt", [C, HID], f32, kind="ExternalInput")
    wot_d = nc.dram_tensor("wot", [HID, C], f32, kind="ExternalInput")
    nm_d = nc.dram_tensor("nmask", [128, 128], f32, kind="ExternalInput")  # 1 - I

    att_d = nc.dram_tensor("att", [TN, MH], f32, kind="ExternalOutput")
    y_d = nc.dram_tensor("y", [C, MH], f32, kind="ExternalOutput")

    # DRAM view of att for the tiled writes: rows k = t*1024 + jl*128 + p.
    att_r = att_d.ap().rearrange("(t j p) m -> t p j m", t=T, j=8, p=128)

    with tile.TileContext(nc) as tc:
        with (
            tc.tile_pool(name="const", bufs=1) as cp,
            tc.tile_pool(name="load", bufs=2) as lp,
            tc.tile_pool(name="bigp", bufs=1) as bp,
            tc.tile_pool(name="stage", bufs=1, space=bass.MemorySpace.PSUM) as stp,
            tc.tile_pool(name="scp", bufs=3, space=bass.MemorySpace.PSUM) as scp,
            tc.tile_pool(name="hvp", bufs=1, space=bass.MemorySpace.PSUM) as hvp,
            tc.tile_pool(name="miscp", bufs=1, space=bass.MemorySpace.PSUM) as mp,
        ):
            # ---------------- persistent SBUF tiles ----------------
            wqt = cp.tile([C, HID], f16, tag="wqt")
            wkvt = cp.tile([C, HID], f16, tag="wkvt")
            wot = cp.tile([HID, C], f32, tag="wot")
            nmask = cp.tile([128, 128], f32, tag="nmask")
            hb = cp.tile([C, N], f16, tag="hb")
            hbs = cp.tile([C, MH], f32, tag="hbs")
            hq = cp.tile([HID, N], f16, tag="hq")
            hkvs = cp.tile([HID, T * MH], f16, tag="hkvs")
            hkvt = cp.tile([128, 64 * (HID + 1)], f16, tag="hkvt")
            ones1 = cp.tile([1, 128], f32, tag="ones1")
            ones1h = cp.tile([1, 128], f16, tag="ones1h")
            one1h = cp.tile([1, 1], f16, tag="one1h")
            big = bp.tile([128, 64 * MH], f16, tag="big")

            # fp32 inputs that stay fp32 ride HWDGE; fp16 ones cast in SWDGE.
            nc.sync.dma_start(wot[:], wot_d.ap())
            nc.sync.dma_start(nmask[:], nm_d.ap())
            nc.sync.dma_start(hbs[:], hbs_d.ap())
            nc.gpsimd.dma_start(wqt[:], wqt_d.ap())
            nc.gpsimd.dma_start(wkvt[:], wkvt_d.ap())
            nc.gpsimd.dma_start(hb[:], hb_d.ap())
            nc.gpsimd.memset(ones1[:], 1.0)
            nc.gpsimd.memset(ones1h[:], 1.0)
            nc.gpsimd.memset(one1h[:], 1.0)

            # ---------------- prologue: h_q ----------------
            for i in range(2):
                st = stp.tile([HID, 512], f32, tag="stage")
                nc.tensor.matmul(st[:], wqt[:],
                                 hb[:, i * 512:(i + 1) * 512])
                nc.scalar.copy(hq[:, i * 512:(i + 1) * 512], st[:])

            # ---------------- prologue: H_kv over the m-slice ----------------
            for q in range(4):
                hsq = lp.tile([C, 1024], f16, tag="hsq")
                nc.gpsimd.dma_start(hsq[:], hs_d.ap()[:, q * 1024:(q + 1) * 1024])
                for tt in range(2):
                    t = 2 * q + tt
                    st = stp.tile([HID, 512], f32, tag="stage")
                    nc.tensor.matmul(st[:], wkvt[:],
                                     hsq[:, tt * 512:(tt + 1) * 512])
                    nc.scalar.copy(hkvs[:, t * 512:(t + 1) * 512], st[:])

            # ---------------- prologue: H_kv^T (augmented) ----------------
            # 8 chunks of [128 k, 32 d + ones col] per stage tile; the ones
            # column is produced in PSUM by a K=1 outer-product matmul so the
            # whole augmented group reaches SBUF in ONE contiguous copy.
            GW = 8 * (HID + 1)  # 264
            for q8 in range(8):
                hfq = lp.tile([C, 1024], f16, tag="hfq")
                nc.gpsimd.dma_start(hfq[:],
                                    hf_d.ap()[:, q8 * 1024:(q8 + 1) * 1024])
                kvst = stp.tile([128, GW], f32, tag="stage")
                for cl in range(8):
                    o = cl * (HID + 1)
                    nc.tensor.matmul(
                        kvst[:, o:o + HID],
                        hfq[:, cl * 128:(cl + 1) * 128],
                        wkvt[:],
                        start=(cl == 0), stop=False)
                    nc.tensor.matmul(
                        kvst[:, o + HID:o + HID + 1],
                        ones1h[:], one1h[:],
                        start=False, stop=(cl == 7))
                nc.vector.tensor_copy(hkvt[:, q8 * GW:(q8 + 1) * GW], kvst[:])

            # ---------------- main passes ----------------
            bcast16, bcast32, hv_ps = {}, {}, {}
            big_r = big[:].rearrange("p (i m) -> p i m", m=MH)

            def emit_pair(c, t, jp):
                """Two scores matmuls (tiles jl=2jp, 2jp+1) into one PSUM
                bank, diag-mask if applicable, one FD=512 exp."""
                sc = scp.tile([128, 2 * W], f32, tag="sc")
                for half in range(2):
                    jl = 2 * jp + half
                    nc.tensor.matmul(
                        sc[:, half * W:(half + 1) * W],
                        hq[:, jl * 128:(jl + 1) * 128],
                        hkvs[:, t * MH + c * W: t * MH + (c + 1) * W],
                        start=(half == 0), stop=(half == 1))
                if jp == c:  # tiles 2c and 2c+1 carry this chunk's diagonal
                    nc.vector.tensor_mul(sc[:, 0:128], sc[:, 0:128], nmask[:])
                    nc.vector.tensor_mul(sc[:, W + 128:W + 256],
                                         sc[:, W + 128:W + 256], nmask[:])
                i0 = t * 8 + 2 * jp
                nc.scalar.activation(
                    big_r[:, i0:i0 + 2, c * W:(c + 1) * W], sc[:], Exp)

            def emit_hv(c, i):
                nc.tensor.matmul(
                    hv_ps[c][:],
                    hkvt[:, i * (HID + 1):(i + 1) * (HID + 1)],
                    big[:, i * MH + c * W: i * MH + (c + 1) * W],
                    start=(i == 0), stop=(i == 63))

            def emit_pass1(c, interleave=None):
                """Scores+exp pairs with lag-2 hv matmuls; optionally
                interleave pass-2 work of the other chunk."""
                hv_ps[c] = hvp.tile([HID + 1, W], f32, tag=f"hv{c}",
                                    name=f"hv{c}")
                pairs = [(t, jp) for t in range(T) for jp in range(4)]
                for k, (t, jp) in enumerate(pairs):
                    emit_pair(c, t, jp)
                    if k >= 2:
                        pt, pjp = pairs[k - 2]
                        emit_hv(c, pt * 8 + 2 * pjp)
                        emit_hv(c, pt * 8 + 2 * pjp + 1)
                    if interleave is not None:
                        interleave(k)
                for (t, jp) in pairs[-2:]:
                    emit_hv(c, t * 8 + 2 * jp)
                    emit_hv(c, t * 8 + 2 * jp + 1)

            def emit_inter(c):
                ssum = cp.tile([1, W], f32, tag="ssum", name=f"ssum{c}")
                nc.vector.tensor_copy(ssum[:], hv_ps[c][HID:HID + 1, :])
                recip = cp.tile([1, W], f32, tag="recip")
                nc.vector.reciprocal_approx_fast(recip[:], ssum[:])
                bc = mp.tile([128, W], f32, tag="bc")
                nc.tensor.matmul(bc[:], ones1[:], recip[:])
                bcast16[c] = cp.tile([128, W], f16, tag="bc16",
                                     name=f"bc16_{c}")
                nc.vector.tensor_copy(bcast16[c][:], bc[:])
                bcast32[c] = cp.tile([128, W], f32, tag="bc32",
                                     name=f"bc32_{c}")
                nc.vector.tensor_copy(bcast32[c][:], bc[:])

            def emit_norm(c, i):
                sl = big[:, i * MH + c * W: i * MH + (c + 1) * W]
                nc.vector.tensor_mul(sl, sl, bcast16[c][:])

            def emit_att_dma(c, t):
                # fp16 -> fp32 cast during SWDGE DMA.
                nc.gpsimd.dma_start(
                    att_r[t][:, :, c * W:(c + 1) * W],
                    big_r[:, t * 8:(t + 1) * 8, c * W:(c + 1) * W])

            def emit_y(c):
                hv_sb = cp.tile([HID, W], f32, tag="hv_sb")
                nc.vector.tensor_mul(hv_sb[:], hv_ps[c][:HID, :],
                                     bcast32[c][:HID, :])
                yac = mp.tile([C, W], f32, tag="y")
                nc.tensor.matmul(yac[:], wot[:], hv_sb[:])
                y_sb = cp.tile([C, W], f32, tag="y_sb")
                nc.vector.tensor_add(y_sb[:], yac[:],
                                     hbs[:, c * W:(c + 1) * W])
                nc.sync.dma_start(y_d.ap()[:, c * W:(c + 1) * W], y_sb[:])

            def make_interleaver(c):
                """Pass-2 of chunk c, spread across the 32 pair-steps of the
                other chunk's pass 1: 2 norms per step + DMA per t."""
                def f(k):
                    emit_norm(c, 2 * k)
                    emit_norm(c, 2 * k + 1)
                    if k % 4 == 3:
                        emit_att_dma(c, k // 4)
                return f

            if MC != 2:
                raise NotImplementedError("kernel is laid out for MC=2")
            emit_pass1(0)
            emit_inter(0)
            emit_pass1(1, interleave=make_interleaver(0))
            emit_y(0)
            emit_inter(1)
            for i in range(64):
                emit_norm(1, i)
                if i % 8 == 7:
                    emit_att_dma(1, i // 8)
            emit_y(1)

    nc.compile()
    return nc


def _shard_inputs(H, h, w_q, w_kv, w_o):
    """Build the 8 per-core input dicts (with the n-rotation trick)."""
    Hf = np.ascontiguousarray(H.reshape(B, C, TN), dtype=np.float32)
    hb = np.ascontiguousarray(h.reshape(B, C, N), dtype=np.float32)
    wqt = np.ascontiguousarray(w_q.T, dtype=np.float32)
    wkvt = np.ascontiguousarray(w_kv.T, dtype=np.float32)
    wot = np.ascontiguousarray(w_o.T, dtype=np.float32)
    nmask = np.ascontiguousarray(1.0 - np.eye(128), dtype=np.float32)

    in_maps = []
    for core in range(NCORES):
        b, hh = divmod(core, 2)
        m0 = hh * MH
        Hb = Hf[b].reshape(C, T, N)
        hf_rot = np.roll(Hb, -m0, axis=2).reshape(C, TN)
        hs = Hb[:, :, m0:m0 + MH].reshape(C, T * MH)
        hb_rot = np.roll(hb[b], -m0, axis=1)
        hbs = hb[b][:, m0:m0 + MH]
        in_maps.append({
            "hf": np.ascontiguousarray(hf_rot),
            "hs": np.ascontiguousarray(hs),
            "hb": np.ascontiguousarray(hb_rot),
            "hbs": np.ascontiguousarray(hbs),
            "wqt": wqt, "wkvt": wkvt, "wot": wot, "nmask": nmask,
        })
    return in_maps


def _assemble(results):
    """Gather per-core outputs into full (y, att), undoing the rotation."""
    y_full = np.empty((B, C, N), dtype=np.float32)
    att_full = np.empty((B, TN, N), dtype=np.float32)
    for core in range(NCORES):
        b, hh = divmod(core, 2)
        m0 = hh * MH
        r = results[core]
        y_full[b][:, m0:m0 + MH] = r["y"]
        a = r["att"].reshape(T, N, MH)
        if m0:
            a = np.roll(a, m0, axis=1)
        att_full[b][:, m0:m0 + MH] = a.reshape(TN, MH)
    return y_full.reshape(B, C, Hh, Ww), att_full


def _run(in_maps, trace=False, **kw):
    from concourse.bass_utils import run_bass_kernel_spmd
    if "prog" not in _cache:
        _cache["prog"] = _build_program()
    nc = _cache["prog"]
    return run_bass_kernel_spmd(nc, in_maps, core_ids=list(range(NCORES)),
                                trace=trace, **kw)


def kernel(H, h, w_q, w_kv, w_o):
    in_maps = _shard_inputs(np.asarray(H), np.asarray(h), np.asarray(w_q),
                            np.asarray(w_kv), np.asarray(w_o))
    res = _run(in_maps)
    return _assemble(res.results)


# revision 30
# speedup vs baseline: 1.7798x; 1.0184x over previous
"""Trainium2 Bass kernel for nn_NonLocAtt (non-local attention block).

Reference computation (per batch b):
    H_kv = w_kv @ H[b]            # [32, 8192]   (k = t*1024 + n)
    h_q  = w_q  @ h[b]            # [32, 1024]
    a[t, n, m] = h_q[:, n] . H_kv[:, t, m]     # scores
    a[t, m, m] = 0                             # zero per-t diagonal
    att = softmax over rows (t, n) of a        # [8192, 1024]
    h_v = H_kv_flat @ att                      # [32, 1024]
    y   = h[b] + w_o @ h_v                     # [64, 1024]
    return (y, att)

Sharding: 8 cores = 4 batches x 2 column-halves (columns m are fully
independent: softmax is over rows, h_v contracts rows).  One SPMD program
serves all cores via the "rotation trick": each core's inputs are
pre-rotated along n by its column offset m0, so the per-t diagonal always
lands in local n-chunks 0..3 at a program-fixed position; the host
un-rotates att rows when assembling the full output.

On-chip layout: rows (t, n) on partitions (tiles of 128), columns m on the
free axis.  The hot matmuls run in fp16 (fp32 matmuls lower to 2 half-rate
passes on TRN2 = 4x slower; fp16 keeps 10 mantissa bits, ~5e-4).  Pass 1
computes exp(scores) (fp16) into an 8 MB SBUF tile; a ones-augmented
H_kv^T matmul accumulates h_v and the softmax column sums together in one
PSUM group.  Pass 2 normalizes in fp16 on DVE (2x mode) and streams att
out via cast-to-f32 SWDGE DMA.  Two column chunks pipeline pass 1 of one
chunk against pass 2 of the other.
"""

import numpy as np

B, C, T, Hh, Ww = 4, 64, 8, 32, 32
HID = 32
N = Hh * Ww          # 1024
TN = T * N           # 8192
MH = N // 2          # 512 columns per core
MC = 2               # column chunks per core (pipeline pass1 vs pass2)
W = MH // MC         # chunk width (256)
NCORES = 8

_cache = {}


def _build_program():
    import concourse.bass as bass
    import concourse.tile as tile
    from concourse import bacc, mybir

    f32 = mybir.dt.float32
    f16 = mybir.dt.float16
    Exp = mybir.ActivationFunctionType.Exp

    nc = bacc.Bacc("TRN2", target_bir_lowering=False, debug=False,
                   num_devices=NCORES)

    # Per-core inputs (host pre-shards / pre-rotates).
    hf_d = nc.dram_tensor("hf", [C, TN], f32, kind="ExternalInput")     # rotated
    hs_d = nc.dram_tensor("hs", [C, T * MH], f32, kind="ExternalInput")  # unrotated m-slice
    hb_d = nc.dram_tensor("hb", [C, N], f32, kind="ExternalInput")      # rotated
    hbs_d = nc.dram_tensor("hbs", [C, MH], f32, kind="ExternalInput")   # unrotated m-slice
    wqt_d = nc.dram_tensor("wqt", [C, HID], f32, kind="ExternalInput")
    wkvt_d = nc.dram_tensor("wkvt", [C, HID], f32, kind="ExternalInput")
    wot_d = nc.dram_tensor("wot", [HID, C], f32, kind="ExternalInput")
    nm_d = nc.dram_tensor("nmask", [128, 128], f32, kind="ExternalInput")  # 1 - I

    att_d = nc.dram_tensor("att", [TN, MH], f32, kind="ExternalOutput")
    y_d = nc.dram_tensor("y", [C, MH], f32, kind="ExternalOutput")

    # DRAM view of att for the tiled writes: rows k = t*1024 + jl*128 + p.
    att_r = att_d.ap().rearrange("(t j p) m -> t p j m", t=T, j=8, p=128)

    with tile.TileContext(nc) as tc:
        with (
            tc.tile_pool(name="const", bufs=1) as cp,
            tc.tile_pool(name="load", bufs=2) as lp,
            tc.tile_pool(name="bigp", bufs=1) as bp,
            tc.tile_pool(name="stage", bufs=1, space=bass.MemorySpace.PSUM) as stp,
            tc.tile_pool(name="scp", bufs=2, space=bass.MemorySpace.PSUM) as scp,
            tc.tile_pool(name="hvp", bufs=1, space=bass.MemorySpace.PSUM) as hvp,
            tc.tile_pool(name="miscp", bufs=1, space=bass.MemorySpace.PSUM) as mp,
        ):
            # ---------------- persistent SBUF tiles ----------------
            wqt = cp.tile([C, HID], f16, tag="wqt")
            wkvt = cp.tile([C, HID], f16, tag="wkvt")
            wot = cp.tile([HID, C], f32, tag="wot")
            nmask = cp.tile([128, 128], f32, tag="nmask")
            hb = cp.tile([C, N], f16, tag="hb")
            hbs = cp.tile([C, MH], f32, tag="hbs")
            hq = cp.tile([HID, N], f16, tag="hq")
            hkvs = cp.tile([HID, T * MH], f16, tag="hkvs")
            hkvt = cp.tile([128, 64 * (HID + 1)], f16, tag="hkvt")
            ones1 = cp.tile([1, 128], f32, tag="ones1")
            ones1h = cp.tile([1, 128], f16, tag="ones1h")
            one1h = cp.tile([1, 1], f16, tag="one1h")
            big = bp.tile([128, 64 * MH], f16, tag="big")

            # fp32 inputs that stay fp32 ride HWDGE; fp16 ones cast in SWDGE.
            nc.sync.dma_start(wot[:], wot_d.ap())
            nc.sync.dma_start(nmask[:], nm_d.ap())
            nc.sync.dma_start(hbs[:], hbs_d.ap())
            nc.gpsimd.dma_start(wqt[:], wqt_d.ap())
            nc.gpsimd.dma_start(wkvt[:], wkvt_d.ap())
            nc.gpsimd.dma_start(hb[:], hb_d.ap())
            nc.gpsimd.memset(ones1[:], 1.0)
            nc.gpsimd.memset(ones1h[:], 1.0)
            nc.gpsimd.memset(one1h[:], 1.0)

            # ---------------- prologue: h_q ----------------
            for i in range(2):
                st = stp.tile([HID, 512], f32, tag="stage")
                nc.tensor.matmul(st[:], wqt[:],
                                 hb[:, i * 512:(i + 1) * 512])
                nc.vector.tensor_copy(hq[:, i * 512:(i + 1) * 512], st[:])

            # ---------------- prologue: H_kv over the m-slice ----------------
            for q in range(4):
                hsq = lp.tile([C, 1024], f16, tag="hsq")
                nc.gpsimd.dma_start(hsq[:], hs_d.ap()[:, q * 1024:(q + 1) * 1024])
                for tt in range(2):
                    t = 2 * q + tt
                    st = stp.tile([HID, 512], f32, tag="stage")
                    nc.tensor.matmul(st[:], wkvt[:],
                                     hsq[:, tt * 512:(tt + 1) * 512])
                    nc.vector.tensor_copy(hkvs[:, t * 512:(t + 1) * 512], st[:])

            # ---------------- prologue: H_kv^T (augmented) ----------------
            # 8 chunks of [128 k, 32 d + ones col] per stage tile; the ones
            # column is produced in PSUM by a K=1 outer-product matmul so the
            # whole augmented group reaches SBUF in ONE contiguous copy.
            GW = 8 * (HID + 1)  # 264
            for q8 in range(8):
                hfq = lp.tile([C, 1024], f16, tag="hfq")
                nc.gpsimd.dma_start(hfq[:],
                                    hf_d.ap()[:, q8 * 1024:(q8 + 1) * 1024])
                kvst = stp.tile([128, GW], f32, tag="stage")
                for cl in range(8):
                    o = cl * (HID + 1)
                    nc.tensor.matmul(
                        kvst[:, o:o + HID],
                        hfq[:, cl * 128:(cl + 1) * 128],
                        wkvt[:],
                        start=(cl == 0), stop=False)
                    nc.tensor.matmul(
                        kvst[:, o + HID:o + HID + 1],
                        ones1h[:], one1h[:],
                        start=False, stop=(cl == 7))
                nc.vector.tensor_copy(hkvt[:, q8 * GW:(q8 + 1) * GW], kvst[:])

            # ---------------- main passes ----------------
            bcast16, bcast32, hv_ps = {}, {}, {}
            big_r = big[:].rearrange("p (i m) -> p i m", m=MH)

            def emit_quad(c, t, jq):
                """Four scores matmuls (tiles jl=4jq..4jq+3) into one 2-bank
                PSUM tile, diag-mask if applicable, one FD=1024 exp."""
                sc = scp.tile([128, 4 * W], f32, tag="sc")
                for half in range(4):
                    jl = 4 * jq + half
                    nc.tensor.matmul(
                        sc[:, half * W:(half + 1) * W],
                        hq[:, jl * 128:(jl + 1) * 128],
                        hkvs[:, t * MH + c * W: t * MH + (c + 1) * W],
                        start=(half % 2 == 0), stop=(half % 2 == 1))
                if jq == 0:  # tiles 2c and 2c+1 carry this chunk's diagonal
                    for k in range(2):
                        lo = (2 * c + k) * W + k * 128
                        nc.vector.tensor_mul(sc[:, lo:lo + 128],
                                             sc[:, lo:lo + 128], nmask[:])
                i0 = t * 8 + 4 * jq
                nc.scalar.activation(
                    big_r[:, i0:i0 + 4, c * W:(c + 1) * W], sc[:], Exp)

            def emit_hv(c, i):
                nc.tensor.matmul(
                    hv_ps[c][:],
                    hkvt[:, i * (HID + 1):(i + 1) * (HID + 1)],
                    big[:, i * MH + c * W: i * MH + (c + 1) * W],
                    start=(i == 0), stop=(i == 63))

            def emit_pass1(c, interleave=None):
                """Scores+exp quads with lag-1 hv matmuls; optionally
                interleave pass-2 work of the other chunk."""
                hv_ps[c] = hvp.tile([HID + 1, W], f32, tag=f"hv{c}",
                                    name=f"hv{c}")
                quads = [(t, jq) for t in range(T) for jq in range(2)]
                for k, (t, jq) in enumerate(quads):
                    emit_quad(c, t, jq)
                    if k >= 1:
                        pt, pjq = quads[k - 1]
                        for h in range(4):
                            emit_hv(c, pt * 8 + 4 * pjq + h)
                    if interleave is not None:
                        interleave(k)
                pt, pjq = quads[-1]
                for h in range(4):
                    emit_hv(c, pt * 8 + 4 * pjq + h)

            def emit_inter(c):
                ssum = cp.tile([1, W], f32, tag="ssum", name=f"ssum{c}")
                nc.vector.tensor_copy(ssum[:], hv_ps[c][HID:HID + 1, :])
                recip = cp.tile([1, W], f32, tag="recip")
                nc.vector.reciprocal_approx_fast(recip[:], ssum[:])
                bc = stp.tile([128, W], f32, tag="stage", name=f"bc_{c}")
                nc.tensor.matmul(bc[:], ones1[:], recip[:])
                bcast16[c] = cp.tile([128, W], f16, tag="bc16",
                                     name=f"bc16_{c}")
                nc.vector.tensor_copy(bcast16[c][:], bc[:])
                bcast32[c] = cp.tile([128, W], f32, tag="bc32",
                                     name=f"bc32_{c}")
                nc.vector.tensor_copy(bcast32[c][:], bc[:])

            def emit_norm(c, i):
                sl = big[:, i * MH + c * W: i * MH + (c + 1) * W]
                nc.vector.tensor_mul(sl, sl, bcast16[c][:])

            def emit_att_dma(c, t):
                # fp16 -> fp32 cast during SWDGE DMA.
                nc.gpsimd.dma_start(
                    att_r[t][:, :, c * W:(c + 1) * W],
                    big_r[:, t * 8:(t + 1) * 8, c * W:(c + 1) * W])

            def emit_y(c):
                hv_sb = cp.tile([HID, W], f32, tag="hv_sb")
                nc.vector.tensor_mul(hv_sb[:], hv_ps[c][:HID, :],
                                     bcast32[c][:HID, :])
                yac = stp.tile([C, W], f32, tag="stage", name=f"yac_{c}")
                nc.tensor.matmul(yac[:], wot[:], hv_sb[:])
                y_sb = cp.tile([C, W], f32, tag="y_sb")
                nc.vector.tensor_add(y_sb[:], yac[:],
                                     hbs[:, c * W:(c + 1) * W])
                nc.sync.dma_start(y_d.ap()[:, c * W:(c + 1) * W], y_sb[:])

            def make_interleaver(c):
                """Pass-2 of chunk c, spread across the 16 quad-steps of the
                other chunk's pass 1: 4 norms per step + DMA per t."""
                def f(k):
                    for h in range(4):
                        emit_norm(c, 4 * k + h)
                    if k % 2 == 1:
                        emit_att_dma(c, k // 2)
                return f

            if MC != 2:
                raise NotImplementedError("kernel is laid out for MC=2")
            emit_pass1(0)
            emit_inter(0)
            emit_pass1(1, interleave=make_interleaver(0))
            emit_y(0)
            emit_inter(1)
            for i in range(64):
                emit_norm(1, i)
                if i % 8 == 7:
                    emit_att_dma(1, i // 8)
            emit_y(1)

    nc.compile()
    return nc


def _shard_inputs(H, h, w_q, w_kv, w_o):
    """Build the 8 per-core input dicts (with the n-rotation trick)."""
    Hf = np.ascontiguousarray(H.reshape(B, C, TN), dtype=np.float32)
    hb = np.ascontiguousarray(h.reshape(B, C, N), dtype=np.float32)
    wqt = np.ascontiguousarray(w_q.T, dtype=np.float32)
    wkvt = np.ascontiguousarray(w_kv.T, dtype=np.float32)
    wot = np.ascontiguousarray(w_o.T, dtype=np.float32)
    nmask = np.ascontiguousarray(1.0 - np.eye(128), dtype=np.float32)

    in_maps = []
    for core in range(NCORES):
        b, hh = divmod(core, 2)
        m0 = hh * MH
        Hb = Hf[b].reshape(C, T, N)
        hf_rot = np.roll(Hb, -m0, axis=2).reshape(C, TN)
        hs = Hb[:, :, m0:m0 + MH].reshape(C, T * MH)
        hb_rot = np.roll(hb[b], -m0, axis=1)
        hbs = hb[b][:, m0:m0 + MH]
        in_maps.append({
            "hf": np.ascontiguousarray(hf_rot),
            "hs": np.ascontiguousarray(hs),
            "hb": np.ascontiguousarray(hb_rot),
            "hbs": np.ascontiguousarray(hbs),
            "wqt": wqt, "wkvt": wkvt, "wot": wot, "nmask": nmask,
        })
    return in_maps


def _assemble(results):
    """Gather per-core outputs into full (y, att), undoing the rotation."""
    y_full = np.empty((B, C, N), dtype=np.float32)
    att_full = np.empty((B, TN, N), dtype=np.float32)
    for core in range(NCORES):
        b, hh = divmod(core, 2)
        m0 = hh * MH
        r = results[core]
        y_full[b][:, m0:m0 + MH] = r["y"]
        a = r["att"].reshape(T, N, MH)
        if m0:
            a = np.roll(a, m0, axis=1)
        att_full[b][:, m0:m0 + MH] = a.reshape(TN, MH)
    return y_full.reshape(B, C, Hh, Ww), att_full


def _run(in_maps, trace=False, **kw):
    from concourse.bass_utils import run_bass_kernel_spmd
    if "prog" not in _cache:
        _cache["prog"] = _build_program()
    nc = _cache["prog"]
    return run_bass_kernel_spmd(nc, in_maps, core_ids=list(range(NCORES)),
                                trace=trace, **kw)


def kernel(H, h, w_q, w_kv, w_o):
    in_maps = _shard_inputs(np.asarray(H), np.asarray(h), np.asarray(w_q),
                            np.asarray(w_kv), np.asarray(w_o))
    res = _run(in_maps)
    return _assemble(res.results)
